# revision 5
# baseline (speedup 1.0000x reference)
"""KAN classifier (dense_mlp) Trainium2 Bass kernel.

Strategy
--------
Data-parallel over 8 NeuronCores: each core handles 1024 rows of the
8192-row batch; weights are replicated.

Per core, each KAN layer is computed as ONE fused matmul over an
expanded feature map of K = 768*9 = 6912 contraction rows:
  block f*9 + 0     : silu(x[f-tile])              (base path)
  block f*9 + 1 + j : Bspline_j(x[f-tile]) * 15.625 (spline path, j=0..7)
with the base/spline weights concatenated (and the spline weights
pre-scaled by 2.5^3/6 on the host) so  h = feats @ W.

B-spline bases use the cardinal closed form (verified vs the Cox-de Boor
recursion in the reference):
  6*B_j(u)  = pa^3 - 4*pb^3,  pa = relu(min(u-j, j+4-u)), pb = relu(pa-1)
evaluated in x-domain (u = 2.5x + 5.5) so that no affine pre-pass is
needed; the 2.5^3 rescale folds into the weights.

Engines:
  - hats A_j = relu(min(x-t_j, t_{j+4}-x)): ScalarE (Abs+Relu, table-free)
    for j < ACT_J, custom DVE op for the rest  -> balances ACT vs DVE.
  - cubes (A^3 - 4*relu(A-0.4)^3): one custom DVE op over all 8 pages.
  - silu = x*sigmoid(x): ScalarE Sigmoid + DVE multiply.
  - gelu(exact erf) = 0.5*h*(1+erf(h/sqrt(2))): ScalarE Erf + custom DVE
    fused multiply-add, reading the PSUM accumulator directly.
  All ScalarE functions used (Abs, Relu, Sigmoid, Erf) live in the single
  `sigmoid_and_others` activation-table set -> one table load, no thrash.

Matmuls (bf16 inputs, fp32 PSUM accumulation):
  layer1: for each 512-batch half, 54 K-blocks x 6 output tiles of
  [128,128]x[128,512]; half 0 runs kb-outer (features stream in
  production order so the PE never starves), half 1 runs o-outer so the
  6 accumulation groups finish staggered and layer-2 work overlaps.
  layer2 (out=2) accumulates 54 [128,2]-weight matmuls into a [2,512]
  PSUM tile per half, interleaved with layer-1 matmuls on the PE.
"""

import numpy as np
import ml_dtypes

B, H, L = 8192, 768, 2
NCORES = 8
BC = B // NCORES          # rows per core
HALF = 512
FT = H // 128             # 6 feature tiles
NKB = FT * 9              # 54 contraction blocks
NB = 8                    # spline coefficients per edge
ACT_J = 4                 # hats produced on ScalarE (rest on VectorE)
STEP = 0.4                # knot step in x domain
KNOT = [float(np.float32(0.4 * j - 2.2)) for j in range(12)]
SPL_SCALE = (2.5 ** 3) / 6.0

_CACHE = {}


def _register_ops():
    if "ops" in _CACHE:
        return _CACHE["ops"]
    import concourse.dve_ops as D
    from concourse.dve_spec import (
        Spec, Src0, Src1, C0, C1, relu, minn, sq, lower, _has_src1,
    )
    from concourse.dve_uop import DveOpSpec

    def mk(name, spec):
        if name in D._SUB_OPCODE_FOR_NAME:
            return next(o for o in D.OPS if o.name == name)
        row = D._CUSTOM_DVE_ROW_BASE + len(D.OPS)
        shas = {}
        for ver in ("v3", "v4"):
            uops = lower(spec, ver=ver)
            shas[ver] = DveOpSpec(
                name=name, opcode=row, uops=uops, rd1_en=_has_src1(spec)
            ).sha(ver)
        op = D.DveOp(name, spec, subdim=False, uops_sha=shas)
        D.OPS.append(op)
        D.CUSTOM_DVE_SPECS[name] = spec
        D._SUB_OPCODE_FOR_NAME[name] = row
        return op

    # A = relu(min(x - s0, s1 - x))   (hat / clamped leg-min)
    hat = mk("KAN_HAT", Spec(
        body=relu(minn(Src0 - C0, C1 - Src0)),
        reference=lambda in0, in1, s0, s1, imm2: np.maximum(
            np.minimum(in0.astype(np.float32) - s0, s1 - in0.astype(np.float32)), 0
        ).astype(np.float32),
    ))

    # out = A^3 - 4*relu(A - s0)^3   (= A*A*A - (2r)^2 * r)
    _s = sq(Src0)
    _m = _s * Src0
    _r = relu(Src0 - C0)
    _rr = _r + _r
    def _cube_ref(in0, in1, s0, s1, imm2):
        a = in0.astype(np.float32)
        r = np.maximum(a - s0, 0)
        return (a ** 3 - 4.0 * r ** 3).astype(np.float32)
    cube = mk("KAN_CUBE", Spec(body=_m - sq(_rr) * _r, reference=_cube_ref))

    # out = (in0*in1 + in0) * s0     (gelu tail: 0.5*h*(1+erf))
    fma1 = mk("KAN_FMA1", Spec(
        body=(Src0 * Src1 + Src0) * C0,
        reference=lambda in0, in1, s0, s1, imm2: (
            (in0.astype(np.float32) * in1 + in0) * s0
        ).astype(np.float32),
    ))

    _CACHE["ops"] = (hat, cube, fma1)
    return _CACHE["ops"]


def _build():
    if "nc" in _CACHE:
        return _CACHE["nc"]
    import concourse.bacc as bacc
    import concourse.mybir as mybir
    from concourse.tile import TileContext

    HAT, CUBE, FMA1 = _register_ops()
    f32, bf16 = mybir.dt.float32, mybir.dt.bfloat16
    AF = mybir.ActivationFunctionType

    nc = bacc.Bacc(
        "TRN2",
        target_bir_lowering=False,
        debug=False,
        enable_asserts=False,
        num_devices=NCORES,
    )
    # Bias constants used by ScalarE activations ([P,1] const APs).
    cvals = {0.8}
    for j in range(ACT_J):
        cvals.add(-0.5 * (KNOT[j] + KNOT[j + 4]))
    for v in sorted(cvals):
        key = (f32, float(v))
        if key not in nc.const_aps.aps:
            t = nc.alloc_sbuf_tensor(f"const-f32-{v}", [128, 1], f32)
            nc.gpsimd.memset(t.ap(), float(v))
            nc.const_aps.aps[key] = t.ap()
    nc.all_engine_barrier()

    xT = nc.dram_tensor("xT", [H, BC], f32, kind="ExternalInput").ap()
    w1a = nc.dram_tensor("w1a", [NKB, 128, H], bf16, kind="ExternalInput").ap()
    w1b = nc.dram_tensor("w1b", [FT, 9, 128, H], bf16, kind="ExternalInput").ap()
    w2 = nc.dram_tensor("w2", [128, NKB * L], bf16, kind="ExternalInput").ap()
    out = nc.dram_tensor("out", [L, BC], f32, kind="ExternalOutput").ap()

    with TileContext(nc) as tc:
        with tc.tile_pool(name="sp", bufs=2) as sp, \
             tc.tile_pool(name="pp", bufs=1, space="PSUM") as pp:

            w2_sb = sp.tile([128, NKB * L], bf16, name="w2_sb", tag="w2_sb", bufs=1)
            nc.sync.dma_start(out=w2_sb[:], in_=w2[:])

            def emit_hats(src_half_ap, a_tile):
                # a_tile: [128, NB, HALF] bf16; src: [128, HALF] f32
                for j in range(ACT_J):
                    c = -0.5 * (KNOT[j] + KNOT[j + 4])
                    s_t = sp.tile([128, HALF], f32, name="S", tag="S")
                    nc.scalar.activation(s_t[:], src_half_ap, AF.Abs, bias=float(c), scale=1.0)
                    nc.scalar.activation(
                        a_tile[:, j, :], s_t[:], AF.Relu, bias=0.8, scale=-1.0
                    )
                for j in range(ACT_J, NB):
                    nc.vector._custom_dve(
                        HAT, out=a_tile[:, j, :], in0=src_half_ap,
                        s0=KNOT[j], s1=KNOT[j + 4],
                    )

            def emit_silu(src_full_ap, dst_bf16):
                sg = sp.tile([128, BC], bf16, name="sg", tag="sg")
                nc.scalar.activation(sg[:], src_full_ap, AF.Sigmoid)
                nc.vector.tensor_mul(out=dst_bf16[:], in0=src_full_ap, in1=sg[:])

            # ---------------- layer-1 features ----------------
            s1, b1, xts = [], [], []
            for f in range(FT):
                xt = sp.tile([128, BC], f32, name="xt", tag="xt", bufs=2)
                nc.sync.dma_start(out=xt[:], in_=xT[f * 128:(f + 1) * 128, :])
                xts.append(xt)
                st = sp.tile([128, BC], bf16, name=f"s1_{f}", tag=f"s1_{f}", bufs=1)
                emit_silu(xt[:], st)
                s1.append(st)
                bt = sp.tile([128, NB, BC], bf16, name=f"b1_{f}", tag=f"b1_{f}", bufs=1)
                for half in range(2):
                    hs = slice(half * HALF, (half + 1) * HALF)
                    ah = sp.tile([128, NB, HALF], bf16, name="Ah", tag="Ah")
                    emit_hats(xt[:, hs], ah)
                    nc.vector._custom_dve(
                        CUBE, out=bt[:, :, hs], in0=ah[:], s0=STEP
                    )
                b1.append(bt)

            def rhs1(kb, hs):
                f, b = divmod(kb, 9)
                return s1[f][:, hs] if b == 0 else b1[f][:, b - 1, hs]

            # layer-2 PSUM accumulators (one per half)
            ps2 = [
                pp.tile([L, HALF], f32, name=f"ps2_{h}", tag="ps2", bufs=2)
                for h in range(2)
            ]
            h2 = [
                sp.tile([128, BC], f32, name=f"h2_{o}", tag=f"h2_{o}", bufs=1)
                for o in range(FT)
            ]

            def mm2(o, blk, rhs_ap, half):
                kb2 = o * 9 + blk
                nc.tensor.matmul(
                    ps2[half][:], w2_sb[:, kb2 * L:(kb2 + 1) * L], rhs_ap,
                    start=(kb2 == 0), stop=(kb2 == NKB - 1),
                )

            def evac_and_l2(o, ps, half):
                hs = slice(half * HALF, (half + 1) * HALF)
                e_t = sp.tile([128, HALF], f32, name="e", tag="e")
                nc.scalar.activation(e_t[:], ps[:], AF.Erf, scale=0.7071067811865476)
                nc.vector._custom_dve(
                    FMA1, out=h2[o][:, hs], in0=ps[:], in1=e_t[:], s0=0.5
                )
                sg2 = sp.tile([128, HALF], f32, name="sg2", tag="S")
                nc.scalar.activation(sg2[:], h2[o][:, hs], AF.Sigmoid)
                s2 = sp.tile([128, HALF], bf16, name="s2", tag="s2")
                nc.vector.tensor_mul(out=s2[:], in0=h2[o][:, hs], in1=sg2[:])
                mm2(o, 0, s2[:], half)
                a2 = sp.tile([128, NB, HALF], bf16, name="A2h", tag="Ah")
                emit_hats(h2[o][:, hs], a2)
                b2 = sp.tile([128, NB, HALF], bf16, name="b2", tag="b2")
                nc.vector._custom_dve(CUBE, out=b2[:], in0=a2[:], s0=STEP)
                for j in range(NB):
                    mm2(o, 1 + j, b2[:, j, :], half)

            # ---------------- layer-1 matmuls, half 0 (kb-outer) -----------
            psA = [
                pp.tile([128, HALF], f32, name=f"psA_{o}", tag="ps", bufs=6)
                for o in range(FT)
            ]
            h0 = slice(0, HALF)
            for kb in range(NKB):
                wrow = sp.tile([128, H], bf16, name="w1a_sb", tag="w1a", bufs=6)
                nc.sync.dma_start(out=wrow[:], in_=w1a[kb])
                r_ap = rhs1(kb, h0)
                for o in range(FT):
                    nc.tensor.matmul(
                        psA[o][:], wrow[:, o * 128:(o + 1) * 128], r_ap,
                        start=(kb == 0), stop=(kb == NKB - 1),
                    )
            for o in range(FT):
                evac_and_l2(o, psA[o], 0)

            # ---------------- layer-1 matmuls, half 1 (o-outer) ------------
            h1 = slice(HALF, BC)
            for o in range(FT):
                psB = pp.tile([128, HALF], f32, name=f"psB_{o}", tag="ps", bufs=6)
                for g in range(9):
                    wg = sp.tile([128, H], bf16, name="w1b_sb", tag="w1b", bufs=4)
                    nc.sync.dma_start(out=wg[:], in_=w1b[o, g])
                    for q in range(6):
                        kb = g * 6 + q
                        nc.tensor.matmul(
                            psB[:], wg[:, q * 128:(q + 1) * 128], rhs1(kb, h1),
                            start=(kb == 0), stop=(kb == NKB - 1),
                        )
                evac_and_l2(o, psB, 1)

            # ---------------- store ----------------
            for half in range(2):
                hs = slice(half * HALF, (half + 1) * HALF)
                ot = sp.tile([L, HALF], f32, name="ot", tag="ot", bufs=1)
                nc.vector.tensor_copy(out=ot[:], in_=ps2[half][:])
                nc.sync.dma_start(out=out[:, hs], in_=ot[:])

    nc.compile()
    _CACHE["nc"] = nc
    return nc


def _wmat(base_w, spline_w, outf):
    """[6912, outf] f32 weight matrix in feature-block row order."""
    base_w = np.asarray(base_w, np.float32)
    spline_w = np.asarray(spline_w, np.float32)
    KB = base_w.T.reshape(FT, 128, outf)
    KS = (spline_w.transpose(1, 2, 0) * np.float32(SPL_SCALE)).reshape(
        FT, 128, NB, outf
    )
    W = np.empty((FT, 9, 128, outf), np.float32)
    W[:, 0] = KB
    W[:, 1:] = KS.transpose(0, 2, 1, 3)
    return W.reshape(FT * 9 * 128, outf)


def _in_maps(hidden, kan_base_w, kan_spline_w, out_base_w, out_spline_w):
    bf = ml_dtypes.bfloat16
    W1 = _wmat(kan_base_w, kan_spline_w, H)
    w1a_h = np.ascontiguousarray(W1.reshape(NKB, 128, H)).astype(bf)
    w1b_h = np.ascontiguousarray(
        W1.reshape(9, 6, 128, FT, 128)
        .transpose(3, 0, 2, 1, 4)
        .reshape(FT, 9, 128, H)
    ).astype(bf)
    W2 = _wmat(out_base_w, out_spline_w, L)
    w2_h = np.ascontiguousarray(
        W2.reshape(NKB, 128, L).transpose(1, 0, 2).reshape(128, NKB * L)
    ).astype(bf)
    xT = np.asarray(hidden, np.float32).T
    return [
        {
            "xT": np.ascontiguousarray(xT[:, c * BC:(c + 1) * BC]),
            "w1a": w1a_h,
            "w1b": w1b_h,
            "w2": w2_h,
        }
        for c in range(NCORES)
    ]


def _run(in_maps, **kw):
    from concourse.bass_utils import run_bass_kernel_spmd
    nc = _build()
    return run_bass_kernel_spmd(nc, in_maps, core_ids=list(range(NCORES)), **kw)


def kernel(hidden, kan_base_w, kan_spline_w, out_base_w, out_spline_w):
    res = _run(_in_maps(hidden, kan_base_w, kan_spline_w, out_base_w, out_spline_w))
    return np.concatenate(
        [r["out"].T for r in res.results], axis=0
    ).astype(np.float32)


# revision 6
# speedup vs baseline: 16403.9804x; 16403.9804x over previous
"""KAN classifier (dense_mlp) Trainium2 Bass kernel.

Strategy
--------
Data-parallel over 8 NeuronCores: each core handles 1024 rows of the
8192-row batch; weights are replicated.

Per core, each KAN layer is computed as ONE fused matmul over an
expanded feature map of K = 768*9 = 6912 contraction rows:
  block f*9 + 0     : silu(x[f-tile])              (base path)
  block f*9 + 1 + j : Bspline_j(x[f-tile]) * 15.625 (spline path, j=0..7)
with the base/spline weights concatenated (and the spline weights
pre-scaled by 2.5^3/6 on the host) so  h = feats @ W.

B-spline bases use the cardinal closed form (verified vs the Cox-de Boor
recursion in the reference):
  6*B_j(u)  = pa^3 - 4*pb^3,  pa = relu(min(u-j, j+4-u)), pb = relu(pa-1)
evaluated in x-domain (u = 2.5x + 5.5) so that no affine pre-pass is
needed; the 2.5^3 rescale folds into the weights.

Engines:
  - hats A_j = relu(min(x-t_j, t_{j+4}-x)): ScalarE (Abs+Relu, table-free)
    for j < ACT_J, custom DVE op for the rest  -> balances ACT vs DVE.
  - cubes (A^3 - 4*relu(A-0.4)^3): one custom DVE op over all 8 pages.
  - silu = x*sigmoid(x): ScalarE Sigmoid + DVE multiply.
  - gelu(exact erf) = 0.5*h*(1+erf(h/sqrt(2))): ScalarE Erf + custom DVE
    fused multiply-add, reading the PSUM accumulator directly.
  All ScalarE functions used (Abs, Relu, Sigmoid, Erf) live in the single
  `sigmoid_and_others` activation-table set -> one table load, no thrash.

Matmuls (bf16 inputs, fp32 PSUM accumulation):
  layer1: for each 512-batch half, 54 K-blocks x 6 output tiles of
  [128,128]x[128,512]; half 0 runs kb-outer (features stream in
  production order so the PE never starves), half 1 runs o-outer so the
  6 accumulation groups finish staggered and layer-2 work overlaps.
  layer2 (out=2) accumulates 54 [128,2]-weight matmuls into a [2,512]
  PSUM tile per half, interleaved with layer-1 matmuls on the PE.
"""

import numpy as np
import ml_dtypes

B, H, L = 8192, 768, 2
NCORES = 8
BC = B // NCORES          # rows per core
HALF = 512
FT = H // 128             # 6 feature tiles
NKB = FT * 9              # 54 contraction blocks
NB = 8                    # spline coefficients per edge
ACT_J = 5                 # hats produced on ScalarE (rest on VectorE)
STEP = 0.4                # knot step in x domain
KNOT = [float(np.float32(0.4 * j - 2.2)) for j in range(12)]
SPL_SCALE = (2.5 ** 3) / 6.0

_CACHE = {}


def _register_ops():
    if "ops" in _CACHE:
        return _CACHE["ops"]
    import concourse.dve_ops as D
    from concourse.dve_spec import (
        Spec, Src0, Src1, C0, C1, relu, minn, sq, lower, _has_src1,
    )
    from concourse.dve_uop import DveOpSpec

    def mk(name, spec):
        if name in D._SUB_OPCODE_FOR_NAME:
            return next(o for o in D.OPS if o.name == name)
        row = D._CUSTOM_DVE_ROW_BASE + len(D.OPS)
        shas = {}
        for ver in ("v3", "v4"):
            uops = lower(spec, ver=ver)
            shas[ver] = DveOpSpec(
                name=name, opcode=row, uops=uops, rd1_en=_has_src1(spec)
            ).sha(ver)
        op = D.DveOp(name, spec, subdim=False, uops_sha=shas)
        D.OPS.append(op)
        D.CUSTOM_DVE_SPECS[name] = spec
        D._SUB_OPCODE_FOR_NAME[name] = row
        return op

    # A = relu(min(x - s0, s1 - x))   (hat / clamped leg-min)
    hat = mk("KAN_HAT", Spec(
        body=relu(minn(Src0 - C0, C1 - Src0)),
        reference=lambda in0, in1, s0, s1, imm2: np.maximum(
            np.minimum(in0.astype(np.float32) - s0, s1 - in0.astype(np.float32)), 0
        ).astype(np.float32),
    ))

    # out = A^3 - 4*relu(A - s0)^3   (= A*A*A - (2r)^2 * r)
    _s = sq(Src0)
    _m = _s * Src0
    _r = relu(Src0 - C0)
    _rr = _r + _r
    def _cube_ref(in0, in1, s0, s1, imm2):
        a = in0.astype(np.float32)
        r = np.maximum(a - s0, 0)
        return (a ** 3 - 4.0 * r ** 3).astype(np.float32)
    cube = mk("KAN_CUBE", Spec(body=_m - sq(_rr) * _r, reference=_cube_ref))

    # out = (in0*in1 + in0) * s0     (gelu tail: 0.5*h*(1+erf))
    fma1 = mk("KAN_FMA1", Spec(
        body=(Src0 * Src1 + Src0) * C0,
        reference=lambda in0, in1, s0, s1, imm2: (
            (in0.astype(np.float32) * in1 + in0) * s0
        ).astype(np.float32),
    ))

    _CACHE["ops"] = (hat, cube, fma1)
    return _CACHE["ops"]


def _build():
    if "nc" in _CACHE:
        return _CACHE["nc"]
    import concourse.bacc as bacc
    import concourse.mybir as mybir
    from concourse.tile import TileContext

    HAT, CUBE, FMA1 = _register_ops()
    f32, bf16 = mybir.dt.float32, mybir.dt.bfloat16
    AF = mybir.ActivationFunctionType

    nc = bacc.Bacc(
        "TRN2",
        target_bir_lowering=False,
        debug=False,
        enable_asserts=False,
        num_devices=NCORES,
    )
    # Bias constants used by ScalarE activations ([P,1] const APs).
    cvals = {0.8}
    for j in range(ACT_J):
        cvals.add(-0.5 * (KNOT[j] + KNOT[j + 4]))
    for v in sorted(cvals):
        key = (f32, float(v))
        if key not in nc.const_aps.aps:
            t = nc.alloc_sbuf_tensor(f"const-f32-{v}", [128, 1], f32)
            nc.gpsimd.memset(t.ap(), float(v))
            nc.const_aps.aps[key] = t.ap()
    nc.all_engine_barrier()

    xT = nc.dram_tensor("xT", [H, BC], f32, kind="ExternalInput").ap()
    w1a = nc.dram_tensor("w1a", [NKB, 128, H], bf16, kind="ExternalInput").ap()
    w1b = nc.dram_tensor("w1b", [FT, 9, 128, H], bf16, kind="ExternalInput").ap()
    w2 = nc.dram_tensor("w2", [128, NKB * L], bf16, kind="ExternalInput").ap()
    out = nc.dram_tensor("out", [L, BC], f32, kind="ExternalOutput").ap()

    with TileContext(nc) as tc:
        with tc.tile_pool(name="sp", bufs=2) as sp, \
             tc.tile_pool(name="pp", bufs=1, space="PSUM") as pp:

            w2_sb = sp.tile([128, NKB * L], bf16, name="w2_sb", tag="w2_sb", bufs=1)
            nc.sync.dma_start(out=w2_sb[:], in_=w2[:])

            def emit_hats(src_half_ap, a_tile):
                # a_tile: [128, NB, HALF] bf16; src: [128, HALF] f32
                for j in range(ACT_J):
                    c = -0.5 * (KNOT[j] + KNOT[j + 4])
                    s_t = sp.tile([128, HALF], f32, name="S", tag="S")
                    nc.scalar.activation(s_t[:], src_half_ap, AF.Abs, bias=float(c), scale=1.0)
                    nc.scalar.activation(
                        a_tile[:, j, :], s_t[:], AF.Relu, bias=0.8, scale=-1.0
                    )
                for j in range(ACT_J, NB):
                    nc.vector._custom_dve(
                        HAT, out=a_tile[:, j, :], in0=src_half_ap,
                        s0=KNOT[j], s1=KNOT[j + 4],
                    )

            def emit_silu(src_full_ap, dst_bf16):
                sg = sp.tile([128, BC], bf16, name="sg", tag="sg")
                nc.scalar.activation(sg[:], src_full_ap, AF.Sigmoid)
                nc.vector.tensor_mul(out=dst_bf16[:], in0=src_full_ap, in1=sg[:])

            # ---------------- layer-1 features ----------------
            s1, b1, xts = [], [], []
            for f in range(FT):
                xt = sp.tile([128, BC], f32, name="xt", tag="xt", bufs=2)
                nc.sync.dma_start(out=xt[:], in_=xT[f * 128:(f + 1) * 128, :])
                xts.append(xt)
                st = sp.tile([128, BC], bf16, name=f"s1_{f}", tag=f"s1_{f}", bufs=1)
                emit_silu(xt[:], st)
                s1.append(st)
                bt = sp.tile([128, NB, BC], bf16, name=f"b1_{f}", tag=f"b1_{f}", bufs=1)
                for half in range(2):
                    hs = slice(half * HALF, (half + 1) * HALF)
                    ah = sp.tile([128, NB, HALF], bf16, name="Ah", tag="Ah")
                    emit_hats(xt[:, hs], ah)
                    nc.vector._custom_dve(
                        CUBE, out=bt[:, :, hs], in0=ah[:], s0=STEP
                    )
                b1.append(bt)

            def rhs1(kb, hs):
                f, b = divmod(kb, 9)
                return s1[f][:, hs] if b == 0 else b1[f][:, b - 1, hs]

            # layer-2 PSUM accumulators (one per half)
            ps2 = [
                pp.tile([L, HALF], f32, name=f"ps2_{h}", tag="ps2", bufs=2)
                for h in range(2)
            ]
            h2 = [
                sp.tile([128, BC], f32, name=f"h2_{o}", tag=f"h2_{o}", bufs=1)
                for o in range(FT)
            ]

            def mm2(o, blk, rhs_ap, half):
                kb2 = o * 9 + blk
                nc.tensor.matmul(
                    ps2[half][:], w2_sb[:, kb2 * L:(kb2 + 1) * L], rhs_ap,
                    start=(kb2 == 0), stop=(kb2 == NKB - 1),
                )

            def evac_and_l2(o, ps, half):
                hs = slice(half * HALF, (half + 1) * HALF)
                e_t = sp.tile([128, HALF], f32, name="e", tag="e")
                nc.scalar.activation(e_t[:], ps[:], AF.Erf, scale=0.7071067811865476)
                nc.vector._custom_dve(
                    FMA1, out=h2[o][:, hs], in0=ps[:], in1=e_t[:], s0=0.5
                )
                sg2 = sp.tile([128, HALF], f32, name="sg2", tag="S")
                nc.scalar.activation(sg2[:], h2[o][:, hs], AF.Sigmoid)
                s2 = sp.tile([128, HALF], bf16, name="s2", tag="s2")
                nc.vector.tensor_mul(out=s2[:], in0=h2[o][:, hs], in1=sg2[:])
                mm2(o, 0, s2[:], half)
                a2 = sp.tile([128, NB, HALF], bf16, name="A2h", tag="Ah")
                emit_hats(h2[o][:, hs], a2)
                b2 = sp.tile([128, NB, HALF], bf16, name="b2", tag="b2")
                nc.vector._custom_dve(CUBE, out=b2[:], in0=a2[:], s0=STEP)
                for j in range(NB):
                    mm2(o, 1 + j, b2[:, j, :], half)

            # ---------------- layer-1 matmuls, half 0 (kb-outer) -----------
            psA = [
                pp.tile([128, HALF], f32, name=f"psA_{o}", tag="ps", bufs=6)
                for o in range(FT)
            ]
            h0 = slice(0, HALF)
            for kb in range(NKB):
                wrow = sp.tile([128, H], bf16, name="w1a_sb", tag="w1a", bufs=6)
                nc.sync.dma_start(out=wrow[:], in_=w1a[kb])
                r_ap = rhs1(kb, h0)
                for o in range(FT):
                    nc.tensor.matmul(
                        psA[o][:], wrow[:, o * 128:(o + 1) * 128], r_ap,
                        start=(kb == 0), stop=(kb == NKB - 1),
                    )
            for o in range(FT):
                evac_and_l2(o, psA[o], 0)

            # ---------------- layer-1 matmuls, half 1 (o-outer) ------------
            h1 = slice(HALF, BC)
            for o in range(FT):
                psB = pp.tile([128, HALF], f32, name=f"psB_{o}", tag="ps", bufs=6)
                for g in range(9):
                    wg = sp.tile([128, H], bf16, name="w1b_sb", tag="w1b", bufs=4)
                    nc.sync.dma_start(out=wg[:], in_=w1b[o, g])
                    for q in range(6):
                        kb = g * 6 + q
                        nc.tensor.matmul(
                            psB[:], wg[:, q * 128:(q + 1) * 128], rhs1(kb, h1),
                            start=(kb == 0), stop=(kb == NKB - 1),
                        )
                evac_and_l2(o, psB, 1)

            # ---------------- store ----------------
            for half in range(2):
                hs = slice(half * HALF, (half + 1) * HALF)
                ot = sp.tile([L, HALF], f32, name="ot", tag="ot", bufs=1)
                nc.vector.tensor_copy(out=ot[:], in_=ps2[half][:])
                nc.sync.dma_start(out=out[:, hs], in_=ot[:])

    nc.compile()
    _CACHE["nc"] = nc
    return nc


def _wmat(base_w, spline_w, outf):
    """[6912, outf] f32 weight matrix in feature-block row order."""
    base_w = np.asarray(base_w, np.float32)
    spline_w = np.asarray(spline_w, np.float32)
    KB = base_w.T.reshape(FT, 128, outf)
    KS = (spline_w.transpose(1, 2, 0) * np.float32(SPL_SCALE)).reshape(
        FT, 128, NB, outf
    )
    W = np.empty((FT, 9, 128, outf), np.float32)
    W[:, 0] = KB
    W[:, 1:] = KS.transpose(0, 2, 1, 3)
    return W.reshape(FT * 9 * 128, outf)


def _in_maps(hidden, kan_base_w, kan_spline_w, out_base_w, out_spline_w):
    bf = ml_dtypes.bfloat16
    W1 = _wmat(kan_base_w, kan_spline_w, H)
    w1a_h = np.ascontiguousarray(W1.reshape(NKB, 128, H)).astype(bf)
    w1b_h = np.ascontiguousarray(
        W1.reshape(9, 6, 128, FT, 128)
        .transpose(3, 0, 2, 1, 4)
        .reshape(FT, 9, 128, H)
    ).astype(bf)
    W2 = _wmat(out_base_w, out_spline_w, L)
    w2_h = np.ascontiguousarray(
        W2.reshape(NKB, 128, L).transpose(1, 0, 2).reshape(128, NKB * L)
    ).astype(bf)
    xT = np.asarray(hidden, np.float32).T
    return [
        {
            "xT": np.ascontiguousarray(xT[:, c * BC:(c + 1) * BC]),
            "w1a": w1a_h,
            "w1b": w1b_h,
            "w2": w2_h,
        }
        for c in range(NCORES)
    ]


def _run(in_maps, **kw):
    from concourse.bass_utils import run_bass_kernel_spmd
    nc = _build()
    return run_bass_kernel_spmd(nc, in_maps, core_ids=list(range(NCORES)), **kw)


def kernel(hidden, kan_base_w, kan_spline_w, out_base_w, out_spline_w):
    res = _run(_in_maps(hidden, kan_base_w, kan_spline_w, out_base_w, out_spline_w))
    return np.concatenate(
        [r["out"].T for r in res.results], axis=0
    ).astype(np.float32)
